# revision 2
# baseline (speedup 1.0000x reference)
"""DCGRU classifier Trainium2 kernel.

Strategy (8 NeuronCores, data-parallel over batch, Bc=4 per core):
  - Whole two-layer DCGRU scan fused on-chip; states live in SBUF across all
    T=128 steps; only per-step input slices are streamed from HBM.
  - bf16 compute / fp32 PSUM accumulation (rel err ~5e-3 vs fp32 reference).
  - Chebyshev fold: gconv mats [x0, x1, 2*S@x1 - x0] @ W == [x0, x1, S@x1] @ W'
    with W'0 = W0 - W2, W'1 = W1, W'2 = 2*W2 (folded on host) -- removes all
    elementwise work in the diffusion recursion.
  - Dual layouts: node-major [n(part), b, feat] for S-contractions,
    feature-major [feat(part), b, n] for the weight contraction; conversions
    via PE transposes and "flipped" matmuls (out = x^T @ S^T directly).
"""

import json

import ml_dtypes
import numpy as np

# ---------------------------------------------------------------------------
# BIR post-pass: this container's walrus rejects >1 sync-wait per instruction
# ("Too many sync wait commands"); split extra waits onto preceding NoOps.
# ---------------------------------------------------------------------------
_MAX_WAITS = 1
_ctr = [0]


def _split_waits(bir: dict) -> dict:
    for fn in bir.get("functions", []):
        for blk in fn.get("blocks", []):
            out = []
            for inst in blk.get("instructions", []):
                si = inst.get("sync_info")
                waits = (si or {}).get("on_wait") or []
                keep = 0 if inst.get("opcode") == "Drain" else _MAX_WAITS
                if len(waits) > keep:
                    extra = waits[: len(waits) - keep]
                    inst["sync_info"]["on_wait"] = waits[len(waits) - keep :]
                    for i in range(0, len(extra), _MAX_WAITS):
                        _ctr[0] += 1
                        out.append(
                            {
                                "debug": inst.get("debug", 0),
                                "engine": inst["engine"],
                                "ins": [],
                                "name": f"I-waitsplit-{_ctr[0]}",
                                "opcode": "NoOp",
                                "outs": [],
                                "sync_info": {
                                    "on_update": [],
                                    "on_wait": extra[i : i + _MAX_WAITS],
                                },
                            }
                        )
                out.append(inst)
            blk["instructions"] = out
    return bir


def _install_waitsplit():
    import concourse.bass as bass

    if getattr(bass.Bass, "_waitsplit_installed", False):
        return
    orig = bass.Bass.to_json_bytes

    def to_json_bytes(self, *a, **k):
        return json.dumps(_split_waits(json.loads(orig(self, *a, **k)))).encode()

    bass.Bass.to_json_bytes = to_json_bytes
    bass.Bass._waitsplit_installed = True


# ---------------------------------------------------------------------------
# Problem constants (hardcoded from the nn_DCGRUClassifier spec)
# ---------------------------------------------------------------------------
B, T, N, IN_DIM = 32, 128, 256, 64
U, K, NCLS = 64, 2, 4
M = K + 1  # 3 Chebyshev mats
N_CORES = 8
Bc = B // N_CORES  # 4
BF = ml_dtypes.bfloat16

UNROLL = 1  # timesteps per For_i iteration (T % UNROLL == 0)


def _fold_w(W: np.ndarray, F: int) -> np.ndarray:
    """W: (F*M, out) with rows indexed (f, m), m fastest. Return [F, M, out]
    bf16 chunks folded for mats [x0, x1, S@x1]."""
    Wm = [W[np.arange(F) * M + m] for m in range(M)]
    chunks = [Wm[0] - Wm[2], Wm[1], 2.0 * Wm[2]]
    return np.stack(chunks, axis=1).astype(BF)  # [F, 3, out]


def build_nc():
    import concourse.bass as bass
    import concourse.mybir as mybir
    import concourse.tile as tile
    from concourse.masks import make_identity

    F32 = mybir.dt.float32
    BF16 = mybir.dt.bfloat16
    AF = mybir.ActivationFunctionType

    nc = bass.Bass("TRN2", target_bir_lowering=False, debug=False, num_devices=N_CORES)

    # ---- DRAM I/O (per core) ----
    d_inp_node = nc.dram_tensor("inp_node", [T, 2, 128, Bc, IN_DIM], BF16, kind="ExternalInput")
    d_inpT = nc.dram_tensor("inpT", [T, IN_DIM, Bc, N], BF16, kind="ExternalInput")
    d_st = nc.dram_tensor("st", [128, 2, N], BF16, kind="ExternalInput")  # S^T chunks
    d_wg = [nc.dram_tensor(f"wg{l}", [128, M, 2 * U], BF16, kind="ExternalInput") for l in (0, 1)]
    d_wc = [nc.dram_tensor(f"wc{l}", [128, M, U], BF16, kind="ExternalInput") for l in (0, 1)]
    d_bg = [nc.dram_tensor(f"bg{l}", [2 * U], F32, kind="ExternalInput") for l in (0, 1)]
    d_bc = [nc.dram_tensor(f"bc{l}", [U], F32, kind="ExternalInput") for l in (0, 1)]
    d_wfc = nc.dram_tensor("wfc", [U, NCLS], BF16, kind="ExternalInput")
    d_bfc = nc.dram_tensor("bfc", [NCLS], F32, kind="ExternalInput")
    d_out = nc.dram_tensor("out", [NCLS, Bc], F32, kind="ExternalOutput")

    with tile.TileContext(nc) as tc:
        with (
            tc.tile_pool(name="const", bufs=1) as cp,
            tc.tile_pool(name="state", bufs=1) as sp,
            tc.tile_pool(name="work", bufs=2) as wp,
            tc.tile_pool(name="ps", bufs=6, space="PSUM") as pp,
        ):
            # ---- constants ----
            ident = cp.tile([128, 128], BF16)
            make_identity(nc, ident)
            sST = cp.tile([128, 2, N], BF16)
            nc.sync.dma_start(out=sST, in_=d_st[:, :, :])
            sWg = [cp.tile([128, M, 2 * U], BF16, name=f"sWg{l}") for l in (0, 1)]
            sWc = [cp.tile([128, M, U], BF16, name=f"sWc{l}") for l in (0, 1)]
            sbg = [cp.tile([128, 1], F32, name=f"sbg{l}") for l in (0, 1)]
            sbc = [cp.tile([U, 1], F32, name=f"sbc{l}") for l in (0, 1)]
            for l in (0, 1):
                nc.sync.dma_start(out=sWg[l], in_=d_wg[l][:, :, :])
                nc.sync.dma_start(out=sWc[l], in_=d_wc[l][:, :, :])
                nc.sync.dma_start(out=sbg[l], in_=d_bg[l][:, None])
                nc.sync.dma_start(out=sbc[l], in_=d_bc[l][:, None])
            sWfc = cp.tile([U, NCLS], BF16)
            nc.sync.dma_start(out=sWfc, in_=d_wfc[:, :])
            sbfc = cp.tile([NCLS, 1], F32)
            nc.sync.dma_start(out=sbfc, in_=d_bfc[:, None])

            # ---- persistent per-layer state ----
            # x0gT: Z-chunk m=0 for gates  = [inpT ; stateT]   (feature-major)
            # x0cT: Z-chunk m=0 for cand   = [inpT ; rstateT]
            # stT:  canonical stateT at partition base 0
            # xcat_node[kc]: node-major [n, b, (inp|state|rstate)] per n-chunk
            x0gT = [sp.tile([128, Bc, N], BF16, name=f"x0gT{l}") for l in (0, 1)]
            x0cT = [sp.tile([128, Bc, N], BF16, name=f"x0cT{l}") for l in (0, 1)]
            stT = [sp.tile([U, Bc, N], BF16, name=f"stT{l}") for l in (0, 1)]
            xnode = [
                [sp.tile([128, Bc, 3 * U], BF16, name=f"xnode{l}_{kc}") for kc in (0, 1)]
                for l in (0, 1)
            ]
            for l in (0, 1):
                nc.vector.memset(stT[l], 0.0)
                nc.vector.memset(x0gT[l][64:128, :, :], 0.0)

            def cell(l, t_inp_node, t_inpT):
                """One DCGRU cell for layer l at the current timestep.
                t_inp_node/t_inpT: DRAM APs for layer-0 input slices (None for l=1).
                """
                xn = xnode[l]
                # -- input halves --
                if l == 0:
                    # DMA input directly into all three destinations
                    nc.sync.dma_start(out=x0gT[0][0:U, :, :], in_=t_inpT)
                    nc.sync.dma_start(out=x0cT[0][0:U, :, :], in_=t_inpT)
                    for kc in (0, 1):
                        nc.sync.dma_start(out=xn[kc][:, :, 0:U], in_=t_inp_node[kc])
                else:
                    # h0T (= stT[0], fresh) -> feature-major dests
                    nc.scalar.copy(out=x0gT[1][0:U, :, :], in_=stT[0])
                    nc.scalar.copy(out=x0cT[1][0:U, :, :], in_=stT[0])
                    # node-major: transpose h0T
                    ph = pp.tile([128, Bc, 2, U], BF16, name="ph_inp", tag="ps")
                    for kc in (0, 1):
                        for b in range(Bc):
                            nc.tensor.transpose(
                                ph[:, b, kc, :],
                                stT[0][:, b, kc * 128 : (kc + 1) * 128],
                                ident[0:U, 0:U],
                            )
                    for kc in (0, 1):
                        nc.vector.tensor_copy(out=xn[kc][:, :, 0:U], in_=ph[:, :, kc, :])

                # -- state -> node-major --
                pst = pp.tile([128, Bc, 2, U], BF16, name="pst", tag="ps")
                for kc in (0, 1):
                    for b in range(Bc):
                        nc.tensor.transpose(
                            pst[:, b, kc, :],
                            stT[l][:, b, kc * 128 : (kc + 1) * 128],
                            ident[0:U, 0:U],
                        )
                for kc in (0, 1):
                    nc.vector.tensor_copy(out=xn[kc][:, :, U : 2 * U], in_=pst[:, :, kc, :])

                # -- x1g = S @ [inp|state]  (node-major out) --
                x1g = [wp.tile([128, Bc, 2 * U], BF16, name=f"x1g{kc}") for kc in (0, 1)]
                for oc in (0, 1):
                    ps = pp.tile([128, Bc, 2 * U], F32, name="ps_x1g", tag="ps")
                    for kc in (0, 1):
                        nc.tensor.matmul(
                            ps,
                            sST[:, kc, oc * 128 : (oc + 1) * 128],
                            xn[kc][:, :, 0 : 2 * U],
                            start=(kc == 0),
                            stop=(kc == 1),
                        )
                    nc.scalar.copy(out=x1g[oc], in_=ps)

                # -- x1gT (feature-major) via PE transposes of x1g --
                pt = pp.tile([128, Bc, 2, 128], BF16, name="pt_x1g", tag="ps")
                for oc in (0, 1):
                    for b in range(Bc):
                        nc.tensor.transpose(pt[:, b, oc, :], x1g[oc][:, b, :], ident)
                x1gT = wp.tile([128, Bc, N], BF16, name="x1gT")
                nc.vector.tensor_copy(out=x1gT, in_=pt[:, :, :, :])

                # -- sx1gT = (S @ x1g)^T via flipped matmuls --
                sx1gT = wp.tile([128, Bc, N], BF16, name="sx1gT")
                for bh in (0, 1):
                    psx = pp.tile([128, 2, N], F32, name="psx_g", tag="ps")
                    for bi in (0, 1):
                        b = 2 * bh + bi
                        for kc in (0, 1):
                            nc.tensor.matmul(
                                psx[:, bi, :],
                                x1g[kc][:, b, :],
                                sST[:, kc, :],
                                start=(kc == 0),
                                stop=(kc == 1),
                            )
                    nc.scalar.copy(out=sx1gT[:, 2 * bh : 2 * bh + 2, :], in_=psx)

                # -- gates = sigmoid(Z_g @ Wg' + bg) --
                Zg = [x0gT[l], x1gT, sx1gT]
                r_sb = wp.tile([U, Bc, N], BF16, name="r_sb")
                u_sb = wp.tile([U, Bc, N], BF16, name="u_sb")
                for h in (0, 1):
                    psg = pp.tile([128, 2 * N], F32, name="ps_gates", tag="ps")
                    for m in range(M):
                        nc.tensor.matmul(
                            psg,
                            sWg[l][:, m, :],
                            Zg[m][:, 2 * h : 2 * h + 2, :],
                            start=(m == 0),
                            stop=(m == M - 1),
                        )
                    nc.scalar.activation(
                        out=r_sb[:, 2 * h : 2 * h + 2, :], in_=psg[0:U, :],
                        func=AF.Sigmoid, bias=sbg[l][0:U, :], scale=1.0,
                    )
                    nc.scalar.activation(
                        out=u_sb[:, 2 * h : 2 * h + 2, :], in_=psg[U:128, :],
                        func=AF.Sigmoid, bias=sbg[l][U:128, :], scale=1.0,
                    )

                # -- rstate = r * state -> x0cT rows U:128 and node-major --
                rs_sb = wp.tile([U, Bc, N], BF16, name="rs_sb")
                nc.vector.tensor_tensor(out=rs_sb, in0=r_sb, in1=stT[l], op=mybir.AluOpType.mult)
                nc.gpsimd.tensor_copy(out=x0cT[l][U:128, :, :], in_=rs_sb)
                prs = pp.tile([128, Bc, 2, U], BF16, name="prs", tag="ps")
                for kc in (0, 1):
                    for b in range(Bc):
                        nc.tensor.transpose(
                            prs[:, b, kc, :],
                            rs_sb[:, b, kc * 128 : (kc + 1) * 128],
                            ident[0:U, 0:U],
                        )
                for kc in (0, 1):
                    nc.vector.tensor_copy(out=xn[kc][:, :, 2 * U : 3 * U], in_=prs[:, :, kc, :])

                # -- x1c = S @ [inp|rstate] (node-major, strided feature pick) --
                x1c = [wp.tile([128, Bc, 2 * U], BF16, name=f"x1c{kc}") for kc in (0, 1)]
                xnv = [
                    xn[kc].rearrange("p b (g c) -> p b g c", c=U) for kc in (0, 1)
                ]
                for oc in (0, 1):
                    ps = pp.tile([128, Bc, 2 * U], F32, name="ps_x1c", tag="ps")
                    for kc in (0, 1):
                        nc.tensor.matmul(
                            ps,
                            sST[:, kc, oc * 128 : (oc + 1) * 128],
                            xnv[kc][:, :, 0:3:2, :],
                            start=(kc == 0),
                            stop=(kc == 1),
                        )
                    nc.scalar.copy(out=x1c[oc], in_=ps)

                # -- x1cT --
                ptc = pp.tile([128, Bc, 2, 128], BF16, name="pt_x1c", tag="ps")
                for oc in (0, 1):
                    for b in range(Bc):
                        nc.tensor.transpose(ptc[:, b, oc, :], x1c[oc][:, b, :], ident)
                x1cT = wp.tile([128, Bc, N], BF16, name="x1cT")
                nc.vector.tensor_copy(out=x1cT, in_=ptc[:, :, :, :])

                # -- sx1cT --
                sx1cT = wp.tile([128, Bc, N], BF16, name="sx1cT")
                for bh in (0, 1):
                    psxc = pp.tile([128, 2, N], F32, name="psx_c", tag="ps")
                    for bi in (0, 1):
                        b = 2 * bh + bi
                        for kc in (0, 1):
                            nc.tensor.matmul(
                                psxc[:, bi, :],
                                x1c[kc][:, b, :],
                                sST[:, kc, :],
                                start=(kc == 0),
                                stop=(kc == 1),
                            )
                    nc.scalar.copy(out=sx1cT[:, 2 * bh : 2 * bh + 2, :], in_=psxc)

                # -- cand = tanh(Z_c @ Wc' + bc) --
                Zc = [x0cT[l], x1cT, sx1cT]
                c_sb = wp.tile([U, Bc, N], BF16, name="c_sb")
                for h in (0, 1):
                    psc = pp.tile([U, 2 * N], F32, name="ps_cand", tag="ps")
                    for m in range(M):
                        nc.tensor.matmul(
                            psc,
                            sWc[l][:, m, :],
                            Zc[m][:, 2 * h : 2 * h + 2, :],
                            start=(m == 0),
                            stop=(m == M - 1),
                        )
                    nc.scalar.activation(
                        out=c_sb[:, 2 * h : 2 * h + 2, :], in_=psc,
                        func=AF.Tanh, bias=sbc[l], scale=1.0,
                    )

                # -- new state = c + u * (state - c) --
                d_sb = wp.tile([U, Bc, N], BF16, name="d_sb")
                nc.vector.tensor_tensor(out=d_sb, in0=stT[l], in1=c_sb, op=mybir.AluOpType.subtract)
                e_sb = wp.tile([U, Bc, N], BF16, name="e_sb")
                nc.vector.tensor_tensor(out=e_sb, in0=u_sb, in1=d_sb, op=mybir.AluOpType.mult)
                nc.vector.tensor_tensor(out=stT[l], in0=c_sb, in1=e_sb, op=mybir.AluOpType.add)
                # state feature-major copy for next step's Z chunk 0
                nc.gpsimd.tensor_copy(out=x0gT[l][U:128, :, :], in_=stT[l])

            # ---- the scan ----
            if UNROLL >= T:
                for t in range(T):
                    cell(0, [d_inp_node[t, kc] for kc in (0, 1)], d_inpT[t])
                    cell(1, None, None)
            else:
                with tc.For_i(0, T, UNROLL) as iv:
                    for j in range(UNROLL):
                        t = iv + j if j else iv
                        cell(0, [d_inp_node[t, kc] for kc in (0, 1)], d_inpT[t])
                        cell(1, None, None)

            # ---- classifier head: max_n(relu(h1) @ Wfc + bfc) ----
            relu_h = wp.tile([U, Bc, N], BF16, name="relu_h")
            nc.scalar.activation(out=relu_h, in_=stT[1], func=AF.Relu)
            ob = wp.tile([NCLS, Bc], F32, name="ob")
            for h in (0, 1):
                pl = pp.tile([NCLS, 2, N], F32, name="ps_log", tag="ps")
                nc.tensor.matmul(
                    pl,
                    sWfc,
                    relu_h[:, 2 * h : 2 * h + 2, :],
                    start=True,
                    stop=True,
                )
                red = wp.tile([NCLS, 2], F32, name="red")
                nc.vector.tensor_reduce(out=red, in_=pl, axis=mybir.AxisListType.X, op=mybir.AluOpType.max)
                nc.vector.tensor_scalar_add(out=ob[:, 2 * h : 2 * h + 2], in0=red, scalar1=sbfc)
            nc.sync.dma_start(out=d_out[:, :], in_=ob)

    return nc


_NC_CACHE = None


def _get_nc():
    global _NC_CACHE
    if _NC_CACHE is None:
        _install_waitsplit()
        _NC_CACHE = build_nc()
    return _NC_CACHE


def kernel(**inputs):
    from concourse.bass_utils import run_bass_kernel_spmd

    nc = _get_nc()

    x = np.asarray(inputs["input_seq"], np.float32)  # (B,T,N,IN)
    S = np.asarray(inputs["supports"], np.float32)

    sST_h = np.ascontiguousarray(S.T.reshape(2, 128, N).transpose(1, 0, 2)).astype(BF)
    wg = [_fold_w(np.asarray(inputs[f"Wg{l}"], np.float32), 128) for l in (0, 1)]
    wc = [_fold_w(np.asarray(inputs[f"Wc{l}"], np.float32), 128) for l in (0, 1)]
    common = {
        "st": sST_h,
        "wg0": wg[0], "wc0": wc[0], "wg1": wg[1], "wc1": wc[1],
        "bg0": np.asarray(inputs["bg0"], np.float32),
        "bc0": np.asarray(inputs["bc0"], np.float32),
        "bg1": np.asarray(inputs["bg1"], np.float32),
        "bc1": np.asarray(inputs["bc1"], np.float32),
        "wfc": np.asarray(inputs["Wfc"], np.float32).astype(BF),
        "bfc": np.asarray(inputs["bfc"], np.float32),
    }
    in_maps = []
    for c in range(N_CORES):
        xc = x[c * Bc : (c + 1) * Bc]  # (Bc, T, N, IN)
        node = np.ascontiguousarray(xc.transpose(1, 2, 0, 3)).reshape(T, 2, 128, Bc, IN_DIM)
        inpT = np.ascontiguousarray(xc.transpose(1, 3, 0, 2))  # (T, IN, Bc, N)
        in_maps.append({**common, "inp_node": node.astype(BF), "inpT": inpT.astype(BF)})

    res = run_bass_kernel_spmd(nc, in_maps, core_ids=list(range(N_CORES)))
    out = np.empty((B, NCLS), np.float32)
    for c in range(N_CORES):
        out[c * Bc : (c + 1) * Bc] = res.results[c]["out"].T
    return out


# revision 5
# speedup vs baseline: 955.5146x; 955.5146x over previous
"""DCGRU classifier Trainium2 kernel.

Strategy (8 NeuronCores, data-parallel over batch, Bc=4 per core):
  - Whole two-layer DCGRU scan fused on-chip; states live in SBUF across all
    T=128 steps; only per-step input slices are streamed from HBM.
  - bf16 compute / fp32 PSUM accumulation (rel err ~5e-3 vs fp32 reference).
  - Chebyshev fold: gconv mats [x0, x1, 2*S@x1 - x0] @ W == [x0, x1, S@x1] @ W'
    with W'0 = W0 - W2, W'1 = W1, W'2 = 2*W2 (folded on host) -- removes all
    elementwise work in the diffusion recursion.
  - Dual layouts: node-major [n(part), b, feat] for S-contractions,
    feature-major [feat(part), b, n] for the weight contraction; conversions
    via PE transposes and "flipped" matmuls (out = x^T @ S^T directly).
"""

import json

import ml_dtypes
import numpy as np

# ---------------------------------------------------------------------------
# BIR post-pass: this container's walrus rejects >1 sync-wait per instruction
# ("Too many sync wait commands"); split extra waits onto preceding NoOps.
# ---------------------------------------------------------------------------
_MAX_WAITS = 1
_ctr = [0]


def _split_waits(bir: dict) -> dict:
    for fn in bir.get("functions", []):
        for blk in fn.get("blocks", []):
            out = []
            for inst in blk.get("instructions", []):
                si = inst.get("sync_info")
                waits = (si or {}).get("on_wait") or []
                keep = 0 if inst.get("opcode") == "Drain" else _MAX_WAITS
                if len(waits) > keep:
                    extra = waits[: len(waits) - keep]
                    inst["sync_info"]["on_wait"] = waits[len(waits) - keep :]
                    for i in range(0, len(extra), _MAX_WAITS):
                        _ctr[0] += 1
                        out.append(
                            {
                                "debug": inst.get("debug", 0),
                                "engine": inst["engine"],
                                "ins": [],
                                "name": f"I-waitsplit-{_ctr[0]}",
                                "opcode": "NoOp",
                                "outs": [],
                                "sync_info": {
                                    "on_update": [],
                                    "on_wait": extra[i : i + _MAX_WAITS],
                                },
                            }
                        )
                out.append(inst)
            blk["instructions"] = out
    return bir


def _install_waitsplit():
    import concourse.bass as bass

    if getattr(bass.Bass, "_waitsplit_installed", False):
        return
    orig = bass.Bass.to_json_bytes

    def to_json_bytes(self, *a, **k):
        return json.dumps(_split_waits(json.loads(orig(self, *a, **k)))).encode()

    bass.Bass.to_json_bytes = to_json_bytes
    bass.Bass._waitsplit_installed = True


# ---------------------------------------------------------------------------
# Problem constants (hardcoded from the nn_DCGRUClassifier spec)
# ---------------------------------------------------------------------------
B, T, N, IN_DIM = 32, 128, 256, 64
U, K, NCLS = 64, 2, 4
M = K + 1  # 3 Chebyshev mats
N_CORES = 8
Bc = B // N_CORES  # 4
BF = ml_dtypes.bfloat16

UNROLL = 1  # timesteps per For_i iteration (T % UNROLL == 0)


def _fold_w(W: np.ndarray, F: int) -> np.ndarray:
    """W: (F*M, out) with rows indexed (f, m), m fastest. Return [F, M, out]
    bf16 chunks folded for mats [x0, x1, S@x1]."""
    Wm = [W[np.arange(F) * M + m] for m in range(M)]
    chunks = [Wm[0] - Wm[2], Wm[1], 2.0 * Wm[2]]
    return np.stack(chunks, axis=1).astype(BF)  # [F, 3, out]


def build_nc(repeats: int = 1):
    import concourse.bass as bass
    import concourse.mybir as mybir
    import concourse.tile as tile
    from concourse.masks import make_identity

    F32 = mybir.dt.float32
    BF16 = mybir.dt.bfloat16
    AF = mybir.ActivationFunctionType

    nc = bass.Bass("TRN2", target_bir_lowering=False, debug=False, num_devices=N_CORES)

    # ---- DRAM I/O (per core) ----
    d_inp_node = nc.dram_tensor("inp_node", [T, 2, 128, Bc, IN_DIM], BF16, kind="ExternalInput")
    d_inpT = nc.dram_tensor("inpT", [T, IN_DIM, Bc, N], BF16, kind="ExternalInput")
    d_st = nc.dram_tensor("st", [128, 2, N], BF16, kind="ExternalInput")  # S^T chunks
    d_wg = [nc.dram_tensor(f"wg{l}", [128, M, 2 * U], BF16, kind="ExternalInput") for l in (0, 1)]
    d_wc = [nc.dram_tensor(f"wc{l}", [128, M, U], BF16, kind="ExternalInput") for l in (0, 1)]
    d_bg = [nc.dram_tensor(f"bg{l}", [2 * U], F32, kind="ExternalInput") for l in (0, 1)]
    d_bc = [nc.dram_tensor(f"bc{l}", [U], F32, kind="ExternalInput") for l in (0, 1)]
    d_wfc = nc.dram_tensor("wfc", [U, NCLS], BF16, kind="ExternalInput")
    d_bfc = nc.dram_tensor("bfc", [NCLS], F32, kind="ExternalInput")
    d_out = nc.dram_tensor("out", [NCLS, Bc], F32, kind="ExternalOutput")

    with tile.TileContext(nc) as tc:
        with (
            tc.tile_pool(name="const", bufs=1) as cp,
            tc.tile_pool(name="state", bufs=1) as sp,
            tc.tile_pool(name="work", bufs=2) as wp,
            tc.tile_pool(name="ps", bufs=6, space="PSUM") as pp,
        ):
            # ---- constants ----
            ident = cp.tile([128, 128], BF16)
            make_identity(nc, ident)
            sST = cp.tile([128, 2, N], BF16)
            nc.sync.dma_start(out=sST, in_=d_st[:, :, :])
            sWg = [cp.tile([128, M, 2 * U], BF16, name=f"sWg{l}") for l in (0, 1)]
            sWc = [cp.tile([128, M, U], BF16, name=f"sWc{l}") for l in (0, 1)]
            sbg = [cp.tile([128, 1], F32, name=f"sbg{l}") for l in (0, 1)]
            sbc = [cp.tile([U, 1], F32, name=f"sbc{l}") for l in (0, 1)]
            for l in (0, 1):
                nc.sync.dma_start(out=sWg[l], in_=d_wg[l][:, :, :])
                nc.sync.dma_start(out=sWc[l], in_=d_wc[l][:, :, :])
                nc.sync.dma_start(out=sbg[l], in_=d_bg[l][:, None])
                nc.sync.dma_start(out=sbc[l], in_=d_bc[l][:, None])
            sWfc = cp.tile([U, NCLS], BF16)
            nc.sync.dma_start(out=sWfc, in_=d_wfc[:, :])
            sbfc = cp.tile([NCLS, 1], F32)
            nc.sync.dma_start(out=sbfc, in_=d_bfc[:, None])

            # ---- persistent per-layer state ----
            # x0gT: Z-chunk m=0 for gates  = [inpT ; stateT]   (feature-major)
            # x0cT: Z-chunk m=0 for cand   = [inpT ; rstateT]
            # stT:  canonical stateT at partition base 0
            # xcat_node[kc]: node-major [n, b, (inp|state|rstate)] per n-chunk
            x0gT = [sp.tile([128, Bc, N], BF16, name=f"x0gT{l}") for l in (0, 1)]
            x0cT = [sp.tile([128, Bc, N], BF16, name=f"x0cT{l}") for l in (0, 1)]
            stT = [sp.tile([U, Bc, N], BF16, name=f"stT{l}") for l in (0, 1)]
            xnode = [
                [sp.tile([128, Bc, 3 * U], BF16, name=f"xnode{l}_{kc}") for kc in (0, 1)]
                for l in (0, 1)
            ]
            def init_and_scan():
                for l in (0, 1):
                    nc.vector.memset(stT[l], 0.0)
                    nc.vector.memset(x0gT[l][64:128, :, :], 0.0)
                run_scan()
                head()

            def cell(l, t_inp_node, t_inpT):
                """One DCGRU cell for layer l at the current timestep.
                t_inp_node/t_inpT: DRAM APs for layer-0 input slices (None for l=1).
                """
                xn = xnode[l]
                # -- input halves --
                if l == 0:
                    # DMA input directly into all three destinations
                    nc.sync.dma_start(out=x0gT[0][0:U, :, :], in_=t_inpT)
                    nc.sync.dma_start(out=x0cT[0][0:U, :, :], in_=t_inpT)
                    for kc in (0, 1):
                        nc.sync.dma_start(out=xn[kc][:, :, 0:U], in_=t_inp_node[kc])
                else:
                    # h0T (= stT[0], fresh) -> feature-major dests
                    nc.scalar.copy(out=x0gT[1][0:U, :, :], in_=stT[0])
                    nc.scalar.copy(out=x0cT[1][0:U, :, :], in_=stT[0])
                    # node-major: transpose h0T
                    ph = pp.tile([128, Bc, 2, U], BF16, name="ph_inp", tag="ps")
                    for kc in (0, 1):
                        for b in range(Bc):
                            nc.tensor.transpose(
                                ph[:, b, kc, :],
                                stT[0][:, b, kc * 128 : (kc + 1) * 128],
                                ident[0:U, 0:U],
                            )
                    for kc in (0, 1):
                        nc.vector.tensor_copy(out=xn[kc][:, :, 0:U], in_=ph[:, :, kc, :])

                # -- state -> node-major --
                pst = pp.tile([128, Bc, 2, U], BF16, name="pst", tag="ps")
                for kc in (0, 1):
                    for b in range(Bc):
                        nc.tensor.transpose(
                            pst[:, b, kc, :],
                            stT[l][:, b, kc * 128 : (kc + 1) * 128],
                            ident[0:U, 0:U],
                        )
                for kc in (0, 1):
                    nc.vector.tensor_copy(out=xn[kc][:, :, U : 2 * U], in_=pst[:, :, kc, :])

                # -- x1g = S @ [inp|state]  (node-major out) --
                x1g = [wp.tile([128, Bc, 2 * U], BF16, name=f"x1g{kc}") for kc in (0, 1)]
                for oc in (0, 1):
                    ps = pp.tile([128, Bc, 2 * U], F32, name="ps_x1g", tag="ps")
                    for kc in (0, 1):
                        nc.tensor.matmul(
                            ps,
                            sST[:, kc, oc * 128 : (oc + 1) * 128],
                            xn[kc][:, :, 0 : 2 * U],
                            start=(kc == 0),
                            stop=(kc == 1),
                        )
                    nc.scalar.copy(out=x1g[oc], in_=ps)

                # -- x1gT (feature-major) via PE transposes of x1g --
                pt = pp.tile([128, Bc, 2, 128], BF16, name="pt_x1g", tag="ps")
                for oc in (0, 1):
                    for b in range(Bc):
                        nc.tensor.transpose(pt[:, b, oc, :], x1g[oc][:, b, :], ident)
                x1gT = wp.tile([128, Bc, N], BF16, name="x1gT")
                nc.vector.tensor_copy(out=x1gT, in_=pt[:, :, :, :])

                # -- sx1gT = (S @ x1g)^T via flipped matmuls --
                sx1gT = wp.tile([128, Bc, N], BF16, name="sx1gT")
                for bh in (0, 1):
                    psx = pp.tile([128, 2, N], F32, name="psx_g", tag="ps")
                    for bi in (0, 1):
                        b = 2 * bh + bi
                        for kc in (0, 1):
                            nc.tensor.matmul(
                                psx[:, bi, :],
                                x1g[kc][:, b, :],
                                sST[:, kc, :],
                                start=(kc == 0),
                                stop=(kc == 1),
                            )
                    nc.scalar.copy(out=sx1gT[:, 2 * bh : 2 * bh + 2, :], in_=psx)

                # -- gates = sigmoid(Z_g @ Wg' + bg) --
                Zg = [x0gT[l], x1gT, sx1gT]
                r_sb = wp.tile([U, Bc, N], BF16, name="r_sb")
                u_sb = wp.tile([U, Bc, N], BF16, name="u_sb")
                for h in (0, 1):
                    psg = pp.tile([128, 2 * N], F32, name="ps_gates", tag="ps")
                    for m in range(M):
                        nc.tensor.matmul(
                            psg,
                            sWg[l][:, m, :],
                            Zg[m][:, 2 * h : 2 * h + 2, :],
                            start=(m == 0),
                            stop=(m == M - 1),
                        )
                    nc.scalar.activation(
                        out=r_sb[:, 2 * h : 2 * h + 2, :], in_=psg[0:U, :],
                        func=AF.Sigmoid, bias=sbg[l][0:U, :], scale=1.0,
                    )
                    nc.scalar.activation(
                        out=u_sb[:, 2 * h : 2 * h + 2, :], in_=psg[U:128, :],
                        func=AF.Sigmoid, bias=sbg[l][U:128, :], scale=1.0,
                    )

                # -- rstate = r * state -> x0cT rows U:128 and node-major --
                rs_sb = wp.tile([U, Bc, N], BF16, name="rs_sb")
                nc.vector.tensor_tensor(out=rs_sb, in0=r_sb, in1=stT[l], op=mybir.AluOpType.mult)
                nc.gpsimd.tensor_copy(out=x0cT[l][U:128, :, :], in_=rs_sb)
                prs = pp.tile([128, Bc, 2, U], BF16, name="prs", tag="ps")
                for kc in (0, 1):
                    for b in range(Bc):
                        nc.tensor.transpose(
                            prs[:, b, kc, :],
                            rs_sb[:, b, kc * 128 : (kc + 1) * 128],
                            ident[0:U, 0:U],
                        )
                for kc in (0, 1):
                    nc.vector.tensor_copy(out=xn[kc][:, :, 2 * U : 3 * U], in_=prs[:, :, kc, :])

                # -- x1c = S @ [inp|rstate] (node-major, strided feature pick) --
                x1c = [wp.tile([128, Bc, 2 * U], BF16, name=f"x1c{kc}") for kc in (0, 1)]
                xnv = [
                    xn[kc].rearrange("p b (g c) -> p b g c", c=U) for kc in (0, 1)
                ]
                for oc in (0, 1):
                    ps = pp.tile([128, Bc, 2 * U], F32, name="ps_x1c", tag="ps")
                    for kc in (0, 1):
                        nc.tensor.matmul(
                            ps,
                            sST[:, kc, oc * 128 : (oc + 1) * 128],
                            xnv[kc][:, :, 0:3:2, :],
                            start=(kc == 0),
                            stop=(kc == 1),
                        )
                    nc.scalar.copy(out=x1c[oc], in_=ps)

                # -- x1cT --
                ptc = pp.tile([128, Bc, 2, 128], BF16, name="pt_x1c", tag="ps")
                for oc in (0, 1):
                    for b in range(Bc):
                        nc.tensor.transpose(ptc[:, b, oc, :], x1c[oc][:, b, :], ident)
                x1cT = wp.tile([128, Bc, N], BF16, name="x1cT")
                nc.vector.tensor_copy(out=x1cT, in_=ptc[:, :, :, :])

                # -- sx1cT --
                sx1cT = wp.tile([128, Bc, N], BF16, name="sx1cT")
                for bh in (0, 1):
                    psxc = pp.tile([128, 2, N], F32, name="psx_c", tag="ps")
                    for bi in (0, 1):
                        b = 2 * bh + bi
                        for kc in (0, 1):
                            nc.tensor.matmul(
                                psxc[:, bi, :],
                                x1c[kc][:, b, :],
                                sST[:, kc, :],
                                start=(kc == 0),
                                stop=(kc == 1),
                            )
                    nc.scalar.copy(out=sx1cT[:, 2 * bh : 2 * bh + 2, :], in_=psxc)

                # -- cand = tanh(Z_c @ Wc' + bc) --
                Zc = [x0cT[l], x1cT, sx1cT]
                c_sb = wp.tile([U, Bc, N], BF16, name="c_sb")
                for h in (0, 1):
                    psc = pp.tile([U, 2 * N], F32, name="ps_cand", tag="ps")
                    for m in range(M):
                        nc.tensor.matmul(
                            psc,
                            sWc[l][:, m, :],
                            Zc[m][:, 2 * h : 2 * h + 2, :],
                            start=(m == 0),
                            stop=(m == M - 1),
                        )
                    nc.scalar.activation(
                        out=c_sb[:, 2 * h : 2 * h + 2, :], in_=psc,
                        func=AF.Tanh, bias=sbc[l], scale=1.0,
                    )

                # -- new state = c + u * (state - c) --
                d_sb = wp.tile([U, Bc, N], BF16, name="d_sb")
                nc.vector.tensor_tensor(out=d_sb, in0=stT[l], in1=c_sb, op=mybir.AluOpType.subtract)
                e_sb = wp.tile([U, Bc, N], BF16, name="e_sb")
                nc.vector.tensor_tensor(out=e_sb, in0=u_sb, in1=d_sb, op=mybir.AluOpType.mult)
                nc.vector.tensor_tensor(out=stT[l], in0=c_sb, in1=e_sb, op=mybir.AluOpType.add)
                # state feature-major copy for next step's Z chunk 0
                nc.gpsimd.tensor_copy(out=x0gT[l][U:128, :, :], in_=stT[l])

            # ---- the scan ----
            def run_scan():
                if UNROLL >= T:
                    for t in range(T):
                        cell(0, [d_inp_node[t, kc] for kc in (0, 1)], d_inpT[t])
                        cell(1, None, None)
                else:
                    with tc.For_i(0, T, UNROLL) as iv:
                        for j in range(UNROLL):
                            t = iv + j if j else iv
                            cell(0, [d_inp_node[t, kc] for kc in (0, 1)], d_inpT[t])
                            cell(1, None, None)

            # ---- classifier head: max_n(relu(h1) @ Wfc + bfc) ----
            def head():
                relu_h = wp.tile([U, Bc, N], BF16, name="relu_h")
                nc.scalar.activation(out=relu_h, in_=stT[1], func=AF.Relu)
                ob = wp.tile([NCLS, Bc], F32, name="ob")
                for h in (0, 1):
                    pl = pp.tile([NCLS, 2, N], F32, name="ps_log", tag="ps")
                    nc.tensor.matmul(
                        pl,
                        sWfc,
                        relu_h[:, 2 * h : 2 * h + 2, :],
                        start=True,
                        stop=True,
                    )
                    red = wp.tile([NCLS, 2], F32, name="red")
                    nc.vector.tensor_reduce(out=red, in_=pl, axis=mybir.AxisListType.X, op=mybir.AluOpType.max)
                    nc.vector.tensor_scalar_add(out=ob[:, 2 * h : 2 * h + 2], in0=red, scalar1=sbfc)
                nc.sync.dma_start(out=d_out[:, :], in_=ob)

            if repeats == 1:
                init_and_scan()
            else:
                with tc.For_i(0, repeats, 1):
                    init_and_scan()

    return nc


_NC_CACHE = None


def _get_nc():
    global _NC_CACHE
    if _NC_CACHE is None:
        _install_waitsplit()
        _NC_CACHE = build_nc()
    return _NC_CACHE


def make_in_maps(inputs):
    x = np.asarray(inputs["input_seq"], np.float32)  # (B,T,N,IN)
    S = np.asarray(inputs["supports"], np.float32)

    sST_h = np.ascontiguousarray(S.T.reshape(2, 128, N).transpose(1, 0, 2)).astype(BF)
    wg = [_fold_w(np.asarray(inputs[f"Wg{l}"], np.float32), 128) for l in (0, 1)]
    wc = [_fold_w(np.asarray(inputs[f"Wc{l}"], np.float32), 128) for l in (0, 1)]
    common = {
        "st": sST_h,
        "wg0": wg[0], "wc0": wc[0], "wg1": wg[1], "wc1": wc[1],
        "bg0": np.asarray(inputs["bg0"], np.float32),
        "bc0": np.asarray(inputs["bc0"], np.float32),
        "bg1": np.asarray(inputs["bg1"], np.float32),
        "bc1": np.asarray(inputs["bc1"], np.float32),
        "wfc": np.asarray(inputs["Wfc"], np.float32).astype(BF),
        "bfc": np.asarray(inputs["bfc"], np.float32),
    }
    in_maps = []
    for c in range(N_CORES):
        xc = x[c * Bc : (c + 1) * Bc]  # (Bc, T, N, IN)
        node = np.ascontiguousarray(xc.transpose(1, 2, 0, 3)).reshape(T, 2, 128, Bc, IN_DIM)
        inpT = np.ascontiguousarray(xc.transpose(1, 3, 0, 2))  # (T, IN, Bc, N)
        in_maps.append({**common, "inp_node": node.astype(BF), "inpT": inpT.astype(BF)})
    return in_maps


def kernel(**inputs):
    from concourse.bass_utils import run_bass_kernel_spmd

    nc = _get_nc()
    in_maps = make_in_maps(inputs)
    res = run_bass_kernel_spmd(nc, in_maps, core_ids=list(range(N_CORES)))
    out = np.empty((B, NCLS), np.float32)
    for c in range(N_CORES):
        out[c * Bc : (c + 1) * Bc] = res.results[c]["out"].T
    return out


# revision 7
# speedup vs baseline: 1220.0325x; 1.2768x over previous
"""DCGRU classifier Trainium2 kernel.

Strategy (8 NeuronCores, data-parallel over batch, Bc=4 per core):
  - Whole two-layer DCGRU scan fused on-chip; states live in SBUF across all
    T=128 steps; only per-step input slices are streamed from HBM.
  - bf16 compute / fp32 PSUM accumulation (rel err ~5e-3 vs fp32 reference).
  - Chebyshev fold: gconv mats [x0, x1, 2*S@x1 - x0] @ W == [x0, x1, S@x1] @ W'
    with W'0 = W0 - W2, W'1 = W1, W'2 = 2*W2 (folded on host) -- removes all
    elementwise work in the diffusion recursion.
  - Dual layouts: node-major [n(part), b, feat] for S-contractions,
    feature-major [feat(part), b, n] for the weight contraction; conversions
    via PE transposes and "flipped" matmuls (out = x^T @ S^T directly).
"""

import json

import ml_dtypes
import numpy as np

# ---------------------------------------------------------------------------
# BIR post-pass: this container's walrus rejects >1 sync-wait per instruction
# ("Too many sync wait commands"); split extra waits onto preceding NoOps.
# ---------------------------------------------------------------------------
_MAX_WAITS = 1
_ctr = [0]


def _split_waits(bir: dict) -> dict:
    for fn in bir.get("functions", []):
        for blk in fn.get("blocks", []):
            out = []
            for inst in blk.get("instructions", []):
                si = inst.get("sync_info")
                waits = (si or {}).get("on_wait") or []
                keep = 0 if inst.get("opcode") == "Drain" else _MAX_WAITS
                if len(waits) > keep:
                    extra = waits[: len(waits) - keep]
                    inst["sync_info"]["on_wait"] = waits[len(waits) - keep :]
                    for i in range(0, len(extra), _MAX_WAITS):
                        _ctr[0] += 1
                        out.append(
                            {
                                "debug": inst.get("debug", 0),
                                "engine": inst["engine"],
                                "ins": [],
                                "name": f"I-waitsplit-{_ctr[0]}",
                                "opcode": "NoOp",
                                "outs": [],
                                "sync_info": {
                                    "on_update": [],
                                    "on_wait": extra[i : i + _MAX_WAITS],
                                },
                            }
                        )
                out.append(inst)
            blk["instructions"] = out
    return bir


def _install_waitsplit():
    import concourse.bass as bass

    if getattr(bass.Bass, "_waitsplit_installed", False):
        return
    orig = bass.Bass.to_json_bytes

    def to_json_bytes(self, *a, **k):
        return json.dumps(_split_waits(json.loads(orig(self, *a, **k)))).encode()

    bass.Bass.to_json_bytes = to_json_bytes
    bass.Bass._waitsplit_installed = True


# ---------------------------------------------------------------------------
# Problem constants (hardcoded from the nn_DCGRUClassifier spec)
# ---------------------------------------------------------------------------
B, T, N, IN_DIM = 32, 128, 256, 64
U, K, NCLS = 64, 2, 4
M = K + 1  # 3 Chebyshev mats
N_CORES = 8
Bc = B // N_CORES  # 4
BF = ml_dtypes.bfloat16

UNROLL = 4  # timesteps per For_i iteration (T % UNROLL == 0)


def _fold_w(W: np.ndarray, F: int) -> np.ndarray:
    """W: (F*M, out) with rows indexed (f, m), m fastest. Return [F, M, out]
    bf16 chunks folded for mats [x0, x1, S@x1]."""
    Wm = [W[np.arange(F) * M + m] for m in range(M)]
    chunks = [Wm[0] - Wm[2], Wm[1], 2.0 * Wm[2]]
    return np.stack(chunks, axis=1).astype(BF)  # [F, 3, out]


def build_nc(repeats: int = 1):
    import concourse.bass as bass
    import concourse.mybir as mybir
    import concourse.tile as tile
    from concourse.masks import make_identity

    F32 = mybir.dt.float32
    BF16 = mybir.dt.bfloat16
    AF = mybir.ActivationFunctionType

    nc = bass.Bass("TRN2", target_bir_lowering=False, debug=False, num_devices=N_CORES)

    # ---- DRAM I/O (per core) ----
    TCH = T // UNROLL
    d_inp_node = nc.dram_tensor("inp_node", [TCH, 2, 128, Bc, UNROLL, IN_DIM], BF16, kind="ExternalInput")
    d_inpT = nc.dram_tensor("inpT", [TCH, IN_DIM, Bc, UNROLL, N], BF16, kind="ExternalInput")
    d_st = nc.dram_tensor("st", [128, 2, N], BF16, kind="ExternalInput")  # S^T chunks
    d_wg = [nc.dram_tensor(f"wg{l}", [128, M, 2 * U], BF16, kind="ExternalInput") for l in (0, 1)]
    d_wc = [nc.dram_tensor(f"wc{l}", [128, M, U], BF16, kind="ExternalInput") for l in (0, 1)]
    d_bg = [nc.dram_tensor(f"bg{l}", [2 * U], F32, kind="ExternalInput") for l in (0, 1)]
    d_bc = [nc.dram_tensor(f"bc{l}", [U], F32, kind="ExternalInput") for l in (0, 1)]
    d_wfc = nc.dram_tensor("wfc", [U, NCLS], BF16, kind="ExternalInput")
    d_bfc = nc.dram_tensor("bfc", [NCLS], F32, kind="ExternalInput")
    d_out = nc.dram_tensor("out", [NCLS, Bc], F32, kind="ExternalOutput")

    with tile.TileContext(nc) as tc:
        with (
            tc.tile_pool(name="const", bufs=1) as cp,
            tc.tile_pool(name="state", bufs=1) as sp,
            tc.tile_pool(name="work", bufs=2) as wp,
            tc.tile_pool(name="ps", bufs=6, space="PSUM") as pp,
        ):
            # ---- constants ----
            ident = cp.tile([128, 128], BF16)
            make_identity(nc, ident)
            sST = cp.tile([128, 2, N], BF16)
            nc.sync.dma_start(out=sST, in_=d_st[:, :, :])
            sWg = [cp.tile([128, M, 2 * U], BF16, name=f"sWg{l}") for l in (0, 1)]
            sWc = [cp.tile([128, M, U], BF16, name=f"sWc{l}") for l in (0, 1)]
            sbg = [cp.tile([128, 1], F32, name=f"sbg{l}") for l in (0, 1)]
            sbc = [cp.tile([U, 1], F32, name=f"sbc{l}") for l in (0, 1)]
            for l in (0, 1):
                nc.sync.dma_start(out=sWg[l], in_=d_wg[l][:, :, :])
                nc.sync.dma_start(out=sWc[l], in_=d_wc[l][:, :, :])
                nc.sync.dma_start(out=sbg[l], in_=d_bg[l][:, None])
                nc.sync.dma_start(out=sbc[l], in_=d_bc[l][:, None])
            sWfc = cp.tile([U, NCLS], BF16)
            nc.sync.dma_start(out=sWfc, in_=d_wfc[:, :])
            sbfc = cp.tile([NCLS, 1], F32)
            nc.sync.dma_start(out=sbfc, in_=d_bfc[:, None])

            # ---- persistent per-layer state ----
            # x0gT: Z-chunk m=0 for gates  = [inpT ; stateT]   (feature-major)
            # x0cT: Z-chunk m=0 for cand   = [inpT ; rstateT]
            # stT:  canonical stateT at partition base 0
            # xcat_node[kc]: node-major [n, b, (inp|state|rstate)] per n-chunk
            x0gT = [sp.tile([128, Bc, N], BF16, name=f"x0gT{l}") for l in (0, 1)]
            x0cT = [sp.tile([128, Bc, N], BF16, name=f"x0cT{l}") for l in (0, 1)]
            stT = [sp.tile([U, Bc, N], BF16, name=f"stT{l}") for l in (0, 1)]
            xnode = [
                [sp.tile([128, Bc, 3 * U], BF16, name=f"xnode{l}_{kc}") for kc in (0, 1)]
                for l in (0, 1)
            ]
            def init_and_scan():
                for l in (0, 1):
                    nc.vector.memset(stT[l], 0.0)
                    nc.vector.memset(x0gT[l][64:128, :, :], 0.0)
                run_scan()
                head()

            def cell(l, t_inp_node, t_inpT):
                """One DCGRU cell for layer l at the current timestep.
                t_inp_node/t_inpT: DRAM APs for layer-0 input slices (None for l=1).
                """
                xn = xnode[l]
                # -- input halves --
                if l == 0:
                    # DMA input directly into all three destinations
                    tc_i, j = t_inpT
                    nc.sync.dma_start(out=x0gT[0][0:U, :, :], in_=d_inpT[tc_i, :, :, j, :])
                    nc.sync.dma_start(out=x0cT[0][0:U, :, :], in_=d_inpT[tc_i, :, :, j, :])
                    for kc in (0, 1):
                        nc.sync.dma_start(out=xn[kc][:, :, 0:U], in_=d_inp_node[tc_i, kc, :, :, j, :])
                else:
                    # h0T (= stT[0], fresh) -> feature-major dests
                    nc.scalar.copy(out=x0gT[1][0:U, :, :], in_=stT[0])
                    nc.scalar.copy(out=x0cT[1][0:U, :, :], in_=stT[0])
                    # node-major: transpose h0T
                    ph = pp.tile([128, Bc, 2, U], BF16, name="ph_inp", tag="ps")
                    for kc in (0, 1):
                        for b in range(Bc):
                            nc.tensor.transpose(
                                ph[:, b, kc, :],
                                stT[0][:, b, kc * 128 : (kc + 1) * 128],
                                ident[0:U, 0:U],
                            )
                    for kc in (0, 1):
                        nc.vector.tensor_copy(out=xn[kc][:, :, 0:U], in_=ph[:, :, kc, :])

                # -- state -> node-major --
                pst = pp.tile([128, Bc, 2, U], BF16, name="pst", tag="ps")
                for kc in (0, 1):
                    for b in range(Bc):
                        nc.tensor.transpose(
                            pst[:, b, kc, :],
                            stT[l][:, b, kc * 128 : (kc + 1) * 128],
                            ident[0:U, 0:U],
                        )
                for kc in (0, 1):
                    nc.vector.tensor_copy(out=xn[kc][:, :, U : 2 * U], in_=pst[:, :, kc, :])

                # -- x1g = S @ [inp|state]  (node-major out) --
                x1g = [wp.tile([128, Bc, 2 * U], BF16, name=f"x1g{kc}") for kc in (0, 1)]
                for oc in (0, 1):
                    ps = pp.tile([128, Bc, 2 * U], F32, name="ps_x1g", tag="ps")
                    for kc in (0, 1):
                        nc.tensor.matmul(
                            ps,
                            sST[:, kc, oc * 128 : (oc + 1) * 128],
                            xn[kc][:, :, 0 : 2 * U],
                            start=(kc == 0),
                            stop=(kc == 1),
                        )
                    nc.scalar.copy(out=x1g[oc], in_=ps)

                # -- x1gT (feature-major) via PE transposes of x1g --
                pt = pp.tile([128, Bc, 2, 128], BF16, name="pt_x1g", tag="ps")
                for oc in (0, 1):
                    for b in range(Bc):
                        nc.tensor.transpose(pt[:, b, oc, :], x1g[oc][:, b, :], ident)
                x1gT = wp.tile([128, Bc, N], BF16, name="x1gT")
                nc.vector.tensor_copy(out=x1gT, in_=pt[:, :, :, :])

                # -- sx1gT = (S @ x1g)^T via flipped matmuls --
                sx1gT = wp.tile([128, Bc, N], BF16, name="sx1gT")
                for bh in (0, 1):
                    psx = pp.tile([128, 2, N], F32, name="psx_g", tag="ps")
                    for bi in (0, 1):
                        b = 2 * bh + bi
                        for kc in (0, 1):
                            nc.tensor.matmul(
                                psx[:, bi, :],
                                x1g[kc][:, b, :],
                                sST[:, kc, :],
                                start=(kc == 0),
                                stop=(kc == 1),
                            )
                    nc.scalar.copy(out=sx1gT[:, 2 * bh : 2 * bh + 2, :], in_=psx)

                # -- gates = sigmoid(Z_g @ Wg' + bg) --
                Zg = [x0gT[l], x1gT, sx1gT]
                r_sb = wp.tile([U, Bc, N], BF16, name="r_sb")
                u_sb = wp.tile([U, Bc, N], BF16, name="u_sb")
                for h in (0, 1):
                    psg = pp.tile([128, 2 * N], F32, name="ps_gates", tag="ps")
                    for m in range(M):
                        nc.tensor.matmul(
                            psg,
                            sWg[l][:, m, :],
                            Zg[m][:, 2 * h : 2 * h + 2, :],
                            start=(m == 0),
                            stop=(m == M - 1),
                        )
                    nc.scalar.activation(
                        out=r_sb[:, 2 * h : 2 * h + 2, :], in_=psg[0:U, :],
                        func=AF.Sigmoid, bias=sbg[l][0:U, :], scale=1.0,
                    )
                    nc.scalar.activation(
                        out=u_sb[:, 2 * h : 2 * h + 2, :], in_=psg[U:128, :],
                        func=AF.Sigmoid, bias=sbg[l][U:128, :], scale=1.0,
                    )

                # -- rstate = r * state -> x0cT rows U:128 and node-major --
                rs_sb = wp.tile([U, Bc, N], BF16, name="rs_sb")
                nc.vector.tensor_tensor(out=rs_sb, in0=r_sb, in1=stT[l], op=mybir.AluOpType.mult)
                nc.gpsimd.tensor_copy(out=x0cT[l][U:128, :, :], in_=rs_sb)
                prs = pp.tile([128, Bc, 2, U], BF16, name="prs", tag="ps")
                for kc in (0, 1):
                    for b in range(Bc):
                        nc.tensor.transpose(
                            prs[:, b, kc, :],
                            rs_sb[:, b, kc * 128 : (kc + 1) * 128],
                            ident[0:U, 0:U],
                        )
                for kc in (0, 1):
                    nc.vector.tensor_copy(out=xn[kc][:, :, 2 * U : 3 * U], in_=prs[:, :, kc, :])

                # -- x1c = S @ [inp|rstate] (node-major, strided feature pick) --
                x1c = [wp.tile([128, Bc, 2 * U], BF16, name=f"x1c{kc}") for kc in (0, 1)]
                xnv = [
                    xn[kc].rearrange("p b (g c) -> p b g c", c=U) for kc in (0, 1)
                ]
                for oc in (0, 1):
                    ps = pp.tile([128, Bc, 2 * U], F32, name="ps_x1c", tag="ps")
                    for kc in (0, 1):
                        nc.tensor.matmul(
                            ps,
                            sST[:, kc, oc * 128 : (oc + 1) * 128],
                            xnv[kc][:, :, 0:3:2, :],
                            start=(kc == 0),
                            stop=(kc == 1),
                        )
                    nc.scalar.copy(out=x1c[oc], in_=ps)

                # -- x1cT --
                ptc = pp.tile([128, Bc, 2, 128], BF16, name="pt_x1c", tag="ps")
                for oc in (0, 1):
                    for b in range(Bc):
                        nc.tensor.transpose(ptc[:, b, oc, :], x1c[oc][:, b, :], ident)
                x1cT = wp.tile([128, Bc, N], BF16, name="x1cT")
                nc.vector.tensor_copy(out=x1cT, in_=ptc[:, :, :, :])

                # -- sx1cT --
                sx1cT = wp.tile([128, Bc, N], BF16, name="sx1cT")
                for bh in (0, 1):
                    psxc = pp.tile([128, 2, N], F32, name="psx_c", tag="ps")
                    for bi in (0, 1):
                        b = 2 * bh + bi
                        for kc in (0, 1):
                            nc.tensor.matmul(
                                psxc[:, bi, :],
                                x1c[kc][:, b, :],
                                sST[:, kc, :],
                                start=(kc == 0),
                                stop=(kc == 1),
                            )
                    nc.scalar.copy(out=sx1cT[:, 2 * bh : 2 * bh + 2, :], in_=psxc)

                # -- cand = tanh(Z_c @ Wc' + bc) --
                Zc = [x0cT[l], x1cT, sx1cT]
                c_sb = wp.tile([U, Bc, N], BF16, name="c_sb")
                for h in (0, 1):
                    psc = pp.tile([U, 2 * N], F32, name="ps_cand", tag="ps")
                    for m in range(M):
                        nc.tensor.matmul(
                            psc,
                            sWc[l][:, m, :],
                            Zc[m][:, 2 * h : 2 * h + 2, :],
                            start=(m == 0),
                            stop=(m == M - 1),
                        )
                    nc.scalar.activation(
                        out=c_sb[:, 2 * h : 2 * h + 2, :], in_=psc,
                        func=AF.Tanh, bias=sbc[l], scale=1.0,
                    )

                # -- new state = c + u * (state - c) --
                d_sb = wp.tile([U, Bc, N], BF16, name="d_sb")
                nc.vector.tensor_tensor(out=d_sb, in0=stT[l], in1=c_sb, op=mybir.AluOpType.subtract)
                e_sb = wp.tile([U, Bc, N], BF16, name="e_sb")
                nc.vector.tensor_tensor(out=e_sb, in0=u_sb, in1=d_sb, op=mybir.AluOpType.mult)
                nc.vector.tensor_tensor(out=stT[l], in0=c_sb, in1=e_sb, op=mybir.AluOpType.add)
                # state feature-major copy for next step's Z chunk 0
                nc.gpsimd.tensor_copy(out=x0gT[l][U:128, :, :], in_=stT[l])

            # ---- the scan ----
            def run_scan():
                if UNROLL >= T:
                    for t in range(T):
                        cell(0, None, (t // UNROLL, t % UNROLL))
                        cell(1, None, None)
                else:
                    with tc.For_i(0, TCH, 1) as iv:
                        for j in range(UNROLL):
                            cell(0, None, (iv, j))
                            cell(1, None, None)

            # ---- classifier head: max_n(relu(h1) @ Wfc + bfc) ----
            def head():
                relu_h = wp.tile([U, Bc, N], BF16, name="relu_h")
                nc.scalar.activation(out=relu_h, in_=stT[1], func=AF.Relu)
                ob = wp.tile([NCLS, Bc], F32, name="ob")
                for h in (0, 1):
                    pl = pp.tile([NCLS, 2, N], F32, name="ps_log", tag="ps")
                    nc.tensor.matmul(
                        pl,
                        sWfc,
                        relu_h[:, 2 * h : 2 * h + 2, :],
                        start=True,
                        stop=True,
                    )
                    red = wp.tile([NCLS, 2], F32, name="red")
                    nc.vector.tensor_reduce(out=red, in_=pl, axis=mybir.AxisListType.X, op=mybir.AluOpType.max)
                    nc.vector.tensor_scalar_add(out=ob[:, 2 * h : 2 * h + 2], in0=red, scalar1=sbfc)
                nc.sync.dma_start(out=d_out[:, :], in_=ob)

            if repeats == 1:
                init_and_scan()
            else:
                with tc.For_i(0, repeats, 1):
                    init_and_scan()

    return nc


_NC_CACHE = None


def _get_nc():
    global _NC_CACHE
    if _NC_CACHE is None:
        _install_waitsplit()
        _NC_CACHE = build_nc()
    return _NC_CACHE


def make_in_maps(inputs):
    x = np.asarray(inputs["input_seq"], np.float32)  # (B,T,N,IN)
    S = np.asarray(inputs["supports"], np.float32)

    sST_h = np.ascontiguousarray(S.T.reshape(2, 128, N).transpose(1, 0, 2)).astype(BF)
    wg = [_fold_w(np.asarray(inputs[f"Wg{l}"], np.float32), 128) for l in (0, 1)]
    wc = [_fold_w(np.asarray(inputs[f"Wc{l}"], np.float32), 128) for l in (0, 1)]
    common = {
        "st": sST_h,
        "wg0": wg[0], "wc0": wc[0], "wg1": wg[1], "wc1": wc[1],
        "bg0": np.asarray(inputs["bg0"], np.float32),
        "bc0": np.asarray(inputs["bc0"], np.float32),
        "bg1": np.asarray(inputs["bg1"], np.float32),
        "bc1": np.asarray(inputs["bc1"], np.float32),
        "wfc": np.asarray(inputs["Wfc"], np.float32).astype(BF),
        "bfc": np.asarray(inputs["bfc"], np.float32),
    }
    in_maps = []
    for c in range(N_CORES):
        xc = x[c * Bc : (c + 1) * Bc]  # (Bc, T, N, IN)
        node = (
            xc.transpose(1, 2, 0, 3)
            .reshape(T // UNROLL, UNROLL, 2, 128, Bc, IN_DIM)
            .transpose(0, 2, 3, 4, 1, 5)
        )
        inpT = (
            xc.transpose(1, 3, 0, 2)
            .reshape(T // UNROLL, UNROLL, IN_DIM, Bc, N)
            .transpose(0, 2, 3, 1, 4)
        )
        in_maps.append({**common, "inp_node": node.astype(BF), "inpT": inpT.astype(BF)})
    return in_maps


def kernel(**inputs):
    from concourse.bass_utils import run_bass_kernel_spmd

    nc = _get_nc()
    in_maps = make_in_maps(inputs)
    res = run_bass_kernel_spmd(nc, in_maps, core_ids=list(range(N_CORES)))
    out = np.empty((B, NCLS), np.float32)
    for c in range(N_CORES):
        out[c * Bc : (c + 1) * Bc] = res.results[c]["out"].T
    return out
